# revision 1
# baseline (speedup 1.0000x reference)
"""GAT layer (edge softmax + weighted scatter) on 8 Trainium2 NeuronCores, v2.

Strategy (dst-range sharding, no collectives):
  - Nodes split into 8 contiguous dst ranges of 6250; dst is sorted, so each
    core owns a contiguous edge range and all of its destination segments.
  - Fixed 32-node window grid (196 windows/core). Edges of each window are
    split by src < 32768 (int16 gather limit) and chunked into <=128-edge
    chunks. Chunk counts per (window, stream) are maxed across cores so all
    8 cores share one compiled schedule; chunks run K=32 per super-step
    (lo-stream supersteps first, then hi).
  - Per super-step: 4x 1024-idx dma_gather pulls raw h[src] rows; ONE
    ap_gather fetches the per-(chunk,window) a_dst vector A[c,w] partition-
    replicated; P = exp(leaky_relu(rowsum(Z*w1) + A)) on the full [128,K,32]
    grid; Sp = onehot(dst_rel) * P is the matmul lhsT, so each chunk's
    scatter matmul accumulates p-weighted rows AND the softmax denominator
    (rhs = [Z*w1 | 1]) into PSUM.
  - Chunks are window-major, so each (window, stream) is one uninterrupted
    start..stop matmul run into a transient [32, 65] PSUM region (bank
    w % 8); the run is then drained with one DVE add into an SBUF
    accumulator acc[32, 196, 65]. Epilogue: divide by the denominator, undo
    the w1 fold, one DMA writes the output.
"""
import sys

sys.path.insert(0, "/opt/trn_rl_repo")

import numpy as np

N, F, E, NCORES = 50000, 64, 800000, 8
NLOC = N // NCORES            # 6250 nodes per core
K = 32                        # chunks per super-step
W = 32                        # window size (dst nodes per chunk)
NPAD = 6272                   # 128 * 49
NWIN = NPAD // W              # 196
HALF = 32768                  # int16 split of the gather table
NEG_SLOPE = 0.01
PC = 192                      # packed f32 cols: 128 idx + 32 drel + 32 aidx
DUMP = NWIN                   # dump window id (pad chunks)


# ---------------------------------------------------------------- host prep
def _wrap16(flat):
    """dma/ap_gather idx layout: idx k at (partition k%16, col k//16),
    replicated across the 8 q7 cores (partition groups of 16)."""
    a = np.asarray(flat, np.int16).reshape(-1, 16).T
    return np.ascontiguousarray(np.tile(a, (8, 1)), dtype=np.int16)


def _prep(src, dst):
    """Split per core / window / stream; find shared per-window chunk counts."""
    cores = []
    for c in range(NCORES):
        n0 = c * NLOC
        e0, e1 = np.searchsorted(dst, [n0, n0 + NLOC])
        s_loc = src[e0:e1].astype(np.int64)
        d_loc = (dst[e0:e1] - n0).astype(np.int64)
        counts = np.bincount(d_loc // W, minlength=NWIN)
        ends = np.cumsum(counts)
        starts = ends - counts
        per_win = []
        for w in range(NWIN):
            sl = slice(starts[w], ends[w])
            s_w, d_w = s_loc[sl], d_loc[sl] - W * w
            m = s_w < HALF
            per_win.append(((s_w[m], d_w[m]), (s_w[~m] - HALF, d_w[~m])))
        cores.append(per_win)

    nch = np.zeros((NWIN, 2), np.int64)
    for per_win in cores:
        for w in range(NWIN):
            for st in (0, 1):
                nch[w, st] = max(nch[w, st],
                                 -(-len(per_win[w][st][0]) // 128))
    nch[nch.sum(1) == 0, 0] = 1      # >=1 chunk per window
    return cores, nch


def _schedule(nch):
    """seq[pos] = (window, chunk_i, start, stop) shared by all cores."""
    runs = []
    for st in (0, 1):
        lst = []
        for w in range(NWIN):
            for i in range(nch[w, st]):
                lst.append((w, i, i == 0, i == nch[w, st] - 1))
        n_sup = -(-len(lst) // K)
        lst += [(DUMP, 0, True, True)] * (n_sup * K - len(lst))
        runs.append(lst)
    s_lo, s_hi = len(runs[0]) // K, len(runs[1]) // K
    return runs[0] + runs[1], s_lo, s_hi


def _build_arrays(per_win, seq, s_lo, s_hi):
    """Per-core packed [S, 128, PC] f32 input."""
    S = s_lo + s_hi
    packed = np.zeros((S, 128, PC), np.float32)
    # pad slots must gather SOME valid row (onehot=0 nullifies them); spread
    # them across the table — row-0 defaults serialize on one HBM bank
    idxg = np.empty((S, 4096), np.int64)
    for s in range(S):
        lim = HALF if s < s_lo else N - HALF
        idxg[s] = (np.arange(4096, dtype=np.int64) * 401 + s * 127) % lim
    drel = np.full((S, 128, K), -1.0, np.float32)
    aidx = np.zeros((S, K * W), np.int64)
    for pos, (w, i, _st, _sp) in enumerate(seq):
        s, c = pos // K, pos % K
        if w == DUMP:
            continue
        st = 0 if s < s_lo else 1
        ss, dd = per_win[w][st]
        ss, dd = ss[128 * i : 128 * i + 128], dd[128 * i : 128 * i + 128]
        ec = len(ss)
        idxg[s, c * 128 : c * 128 + ec] = ss
        drel[s, :ec, c] = dd
        aidx[s, c * W : (c + 1) * W] = W * w + np.arange(W)
    for s in range(S):
        packed[s, :, 0:128] = _wrap16(idxg[s]).view(np.float32)
        packed[s, :, 128:160] = drel[s]
        packed[s, :, 160:192] = _wrap16(aidx[s]).view(np.float32)
    return packed


# ------------------------------------------------------------- bass program
def _build_program(s_lo, s_hi, seq):
    import concourse.bacc as bacc
    import concourse.tile as tile
    import concourse.mybir as mybir
    from concourse import bass

    f32, i16 = mybir.dt.float32, mybir.dt.int16
    AF = mybir.ActivationFunctionType
    OP = mybir.AluOpType
    S = s_lo + s_hi

    nc = bacc.Bacc("TRN2", target_bir_lowering=False, debug=False,
                   num_devices=NCORES, num_swdge_queues=4)
    h_t = nc.dram_tensor("h", [N, F], f32, kind="ExternalInput")
    hs_t = nc.dram_tensor("h_slice", [NPAD, F], f32, kind="ExternalInput")
    w_t = nc.dram_tensor("attn_w", [2 * F], f32, kind="ExternalInput")
    pk_t = nc.dram_tensor("packed", [S, 128, PC], f32, kind="ExternalInput")
    aw_t = nc.dram_tensor("aw", [128, S * K // 32], f32, kind="ExternalInput")
    out_t = nc.dram_tensor("out", [NPAD, F], f32, kind="ExternalOutput")
    adr_t = nc.dram_tensor("adr", [NPAD], f32, kind="Internal")
    ta_t = nc.dram_tensor("ta", [NWIN + 1, F], f32, kind="Internal")
    a_all_t = nc.dram_tensor("a_all", [S * K, W], f32, kind="Internal")

    def bc_ap(tensor, offset, ap):
        return bass.AP(tensor=tensor, offset=offset, ap=ap)

    with tile.TileContext(nc) as tc:
        with tc.tile_pool(name="const", bufs=1) as const, \
             tc.tile_pool(name="pre", bufs=1) as pre, \
             tc.tile_pool(name="ps", bufs=1, space="PSUM") as ps:

            # ---------------- constants
            w1t = const.tile([128, F], f32)
            nc.gpsimd.dma_start(out=w1t[:], in_=bc_ap(w_t, 0, [[0, 128], [1, F]]))
            w2t = const.tile([128, F], f32)
            nc.gpsimd.dma_start(out=w2t[:], in_=bc_ap(w_t, F, [[0, 128], [1, F]]))
            iota32 = const.tile([128, W], f32)
            nc.gpsimd.iota(iota32[:], pattern=[[1, W]], base=0,
                           channel_multiplier=0,
                           allow_small_or_imprecise_dtypes=True)
            rw1 = const.tile([128, F], f32)
            nc.vector.reciprocal(rw1[:], w1t[:])

            # ---------------- preamble: a_dst table + all A rows. The A rows
            # (a_dst values per chunk window) are fetched with ONE dma_gather
            # from a window-row table — same ucode library as the Z gathers,
            # so no gpsimd library reload, and descriptor-rate fast.
            with tc.tile_pool(name="pre2", bufs=1) as pre2:
                hs = pre2.tile([128, NPAD // 128, F], f32)
                nc.sync.dma_start(
                    out=hs[:], in_=hs_t[:].rearrange("(p t) f -> p t f", p=128))
                nc.vector.tensor_tensor(
                    out=hs[:], in0=hs[:],
                    in1=w2t[:, None, :].to_broadcast([128, NPAD // 128, F]),
                    op=OP.mult)
                a_sb = pre2.tile([128, NPAD // 128], f32)
                nc.vector.tensor_reduce(out=a_sb[:], in_=hs[:],
                                        axis=mybir.AxisListType.X, op=OP.add)
                nc.sync.dma_start(
                    out=adr_t[:].rearrange("(p t) -> p t", p=128), in_=a_sb[:])
                a_bc = pre2.tile([128, NPAD], f32)
                nc.sync.dma_start(out=a_bc[:],
                                  in_=bc_ap(adr_t, 0, [[0, 128], [1, NPAD]]))
                # ta_t row w = a_dst[32w .. 32w+32] (cols 32:64 unused);
                # row NWIN = zeros (dump chunks)
                nc.sync.dma_start(
                    out=ta_t[0:NWIN, 0:W],
                    in_=a_bc[0:1, :].rearrange("p (w j) -> p w j", j=W))
                zrow = pre2.tile([1, W], f32)
                nc.vector.memset(zrow[:], 0.0)
                nc.sync.dma_start(out=ta_t[NWIN : NWIN + 1, 0:W],
                                  in_=zrow[:])
                awi = pre2.tile([128, S * K // 32], f32)
                nc.sync.dma_start(out=awi[:], in_=aw_t[:])
                At = pre2.tile([128, S * K // 128, F], f32)
                nc.gpsimd.dma_gather(
                    out_ap=At[:], in_ap=ta_t[:],
                    idxs_ap=awi[:].bitcast(i16), num_idxs=S * K,
                    num_idxs_reg=S * K, elem_size=F, queue_num=0)
                nc.sync.dma_start(
                    out=a_all_t[:].rearrange("(c p) w -> p c w", p=128),
                    in_=At[:, :, 0:W])

            # ---------------- super-steps (2-stage software pipeline)
            # stage A(s): gathers + DVE score prep + scalar exp
            # stage B(s): Sp *= Pm, scatter matmuls, psum drains — emitted
            # one iteration later so the scalar round trip never stalls DVE.
            from contextlib import ExitStack
            lctx = ExitStack()
            accp = lctx.enter_context(tc.tile_pool(name="acc", bufs=1))
            ldi = lctx.enter_context(tc.tile_pool(name="ldi", bufs=6))
            zp = lctx.enter_context(tc.tile_pool(name="zp", bufs=4))
            b3 = lctx.enter_context(tc.tile_pool(name="b3", bufs=3))
            med = lctx.enter_context(tc.tile_pool(name="med", bufs=3))

            # accumulator + transient psum banks
            acc = accp.tile([32, NWIN, F + 1], f32)
            nc.vector.memset(acc[:], 0.0)
            banks = [ps.tile([128, 512], f32, name=f"bank{b}", tag=f"bank{b}")
                     for b in range(8)]

            def bank_region(w):
                return banks[w % 8][0:32, 0:65]

            stash = {}
            for it in range(S + 1):
                if it < S:
                    s = it
                    tab = h_t[0:HALF, :] if s < s_lo else h_t[HALF:N, :]
                    ld = ldi.tile([128, 160], f32, tag="ld")
                    nc.sync.dma_start(out=ld[:], in_=pk_t[s, :, 0:160])
                    ig = ld[:, 0:128].bitcast(i16)

                    Z = zp.tile([128, K, F], f32, tag="Z")
                    for q in range(4):
                        nc.gpsimd.dma_gather(
                            out_ap=Z[:, 8 * q : 8 * q + 8, :],
                            in_ap=tab,
                            idxs_ap=ig[:, 64 * q : 64 * q + 64],
                            num_idxs=1024, num_idxs_reg=1024, elem_size=F,
                            queue_num=q)

                    # A[c, w] = a_dst[32*w_c + w], partition-replicated
                    A = med.tile([128, K, W], f32, tag="A")
                    nc.sync.dma_start(
                        out=A[:],
                        in_=bc_ap(a_all_t, s * K * W,
                                  [[0, 128], [W, K], [1, W]]))

                    # rhs = [Z*w1 | 1]; s_e = rowsum(Z*w1)
                    rhsT = b3.tile([128, K, F + 1], f32, tag="rhsT")
                    nc.vector.memset(rhsT[:, :, F : F + 1], 1.0)
                    nc.vector.tensor_tensor(
                        out=rhsT[:, :, 0:F], in0=Z[:],
                        in1=w1t[:, None, :].to_broadcast([128, K, F]),
                        op=OP.mult)
                    sC = med.tile([128, K], f32, tag="sC")
                    nc.vector.tensor_reduce(out=sC[:], in_=rhsT[:, :, 0:F],
                                            axis=mybir.AxisListType.X,
                                            op=OP.add)

                    # E = leaky_relu(s + A); Sp = onehot(drel) (pre-Pm)
                    Emat = med.tile([128, K, W], f32, tag="Emat")
                    nc.vector.tensor_tensor(
                        out=Emat[:],
                        in0=sC[:, :, None].to_broadcast([128, K, W]),
                        in1=A[:], op=OP.add)
                    Pm = b3.tile([128, K, W], f32, tag="Pm")
                    nc.vector.tensor_scalar_mul(Pm[:], Emat[:], NEG_SLOPE)
                    nc.vector.tensor_tensor(out=Emat[:], in0=Emat[:],
                                            in1=Pm[:], op=OP.max)
                    Sp = b3.tile([128, K, W], f32, tag="Sp")
                    nc.vector.tensor_tensor(
                        out=Sp[:],
                        in0=ld[:, 128:160][:, :, None].to_broadcast(
                            [128, K, W]),
                        in1=iota32[:, None, :].to_broadcast([128, K, W]),
                        op=OP.is_equal)
                    nc.scalar.activation(out=Pm[:], in_=Emat[:], func=AF.Exp)
                    stash[s] = (rhsT, Pm, Sp)

                if it >= 1:
                    s = it - 1
                    rhsT, Pm, Sp = stash.pop(s)
                    nc.vector.tensor_tensor(out=Sp[:], in0=Sp[:], in1=Pm[:],
                                            op=OP.mult)
                    for c in range(K):
                        w, _i, st, sp = seq[s * K + c]
                        reg = bank_region(w)
                        nc.tensor.matmul(out=reg, lhsT=Sp[:, c, :],
                                         rhs=rhsT[:, c, :], start=st, stop=sp)
                        if sp and w != DUMP:
                            nc.vector.tensor_tensor(
                                out=acc[:, w, :], in0=acc[:, w, :], in1=reg,
                                op=OP.add)

            # ---------------- epilogue: divide by r, undo w1 fold (in place)
            rmax = pre.tile([32, NWIN], f32)
            nc.vector.tensor_scalar_max(rmax[:], acc[:, :, F], 1e-30)
            rcp = pre.tile([32, NWIN], f32)
            nc.vector.reciprocal(rcp[:], rmax[:])
            nc.vector.tensor_tensor(
                out=acc[:, :, 0:F], in0=acc[:, :, 0:F],
                in1=rcp[:, :, None].to_broadcast([32, NWIN, F]), op=OP.mult)
            nc.vector.tensor_tensor(
                out=acc[:, :, 0:F], in0=acc[:, :, 0:F],
                in1=rw1[0:32, None, :].to_broadcast([32, NWIN, F]), op=OP.mult)
            nc.sync.dma_start(
                out=out_t[:].rearrange("(t p) f -> p t f", p=32),
                in_=acc[:, :, 0:F])
            lctx.close()
    nc.compile()
    return nc


_prog_cache = {}
_last_in_maps = None


def kernel(h, attn_w, src, dst):
    from concourse.bass_utils import run_bass_kernel_spmd

    h = np.ascontiguousarray(h, dtype=np.float32)
    attn_w = np.ascontiguousarray(attn_w, dtype=np.float32)
    src = np.asarray(src, dtype=np.int32)
    dst = np.asarray(dst, dtype=np.int32)

    cores, nch = _prep(src, dst)
    seq, s_lo, s_hi = _schedule(nch)

    key = (s_lo, s_hi, tuple(seq))
    if key not in _prog_cache:
        _prog_cache[key] = _build_program(s_lo, s_hi, seq)
    nc = _prog_cache[key]

    aw = _wrap16([w for (w, _i, _st, _sp) in seq]).view(np.float32)
    in_maps = []
    for d in range(NCORES):
        n0 = d * NLOC
        packed = _build_arrays(cores[d], seq, s_lo, s_hi)
        h_slice = np.zeros((NPAD, F), np.float32)
        h_slice[:NLOC] = h[n0 : n0 + NLOC]
        in_maps.append({
            "h": h,
            "h_slice": h_slice,
            "attn_w": attn_w,
            "packed": packed,
            "aw": aw,
        })

    global _last_in_maps
    _last_in_maps = in_maps
    res = run_bass_kernel_spmd(nc, in_maps, list(range(NCORES)))
    out = np.concatenate([res.results[d]["out"][:NLOC] for d in range(NCORES)])
    return out.astype(np.float32)


if __name__ == "__main__":
    import reference

    inputs = reference.setup_inputs()
    inputs = {k: np.asarray(v) for k, v in inputs.items()}
    got = kernel(**inputs)
    exp = np.asarray(reference.reference(**inputs))
    denom = np.abs(exp).max()
    rel = np.abs(got - exp).max() / denom
    print("Relative error:", rel)



# revision 13
# speedup vs baseline: 1.1567x; 1.1567x over previous
"""GAT layer (edge softmax + weighted scatter) on 8 Trainium2 NeuronCores, v3.

Strategy (dst-range sharding, no collectives):
  - Nodes split into 8 contiguous dst ranges of 6250; dst is sorted, so each
    core owns a contiguous edge range and all of its destination segments.
  - Fixed 32-node window grid (196 windows/core). Edges of each window are
    split by src < 32768 (int16 gather limit) and chunked into <=128-edge
    chunks. Chunk counts per (window, stream) are maxed across cores so all
    8 cores share one compiled schedule; chunks run K=32 per super-step
    (lo-stream supersteps first, then hi).
  - Gather table is bf16 [N, 128]: row n = [h[n] (64) | 1.0 | zeros(63)].
    Col 64 provides the softmax-denominator ones column for free; bf16 makes
    the scatter matmuls single-pass (fp32 double-pumps the PE array).
  - Per super-step: 4x 1024-idx dma_gather pulls bf16 rows; scores
    e = rowsum(Z*w1) on DVE (bf16 mult + reduce); E = e + a_dst[window cols]
    (a_dst tile broadcast from a preamble-built table); leaky_relu and exp
    run on the Scalar engine (Lrelu + Exp); Sp = P * onehot-mask where the
    mask is HOST-built metadata DMA'd per super-step (no is_equal on DVE).
  - Scatter: matmul lhsT=Sp[:,c,:] [128,32] bf16, rhs=Z[:,c,0:65] bf16 into
    a PSUM-RESIDENT accumulator: 196 windows live across 7 PSUM banks
    (4 partition-groups x 7 col-groups of [32,65] each); start only on a
    window's first chunk, stop on its last. No per-run drains.
  - Epilogue: 7 whole-bank Scalar-engine drains to SBUF, divide features by
    the denominator column, one DMA writes the (window-permuted) output;
    the host inverse-permutes rows.
"""
import sys

sys.path.insert(0, "/opt/trn_rl_repo")

import numpy as np
import ml_dtypes

BF16 = ml_dtypes.bfloat16

N, F, E, NCORES = 50000, 64, 800000, 8
NLOC = N // NCORES            # 6250 nodes per core
K = 32                        # chunks per super-step
W = 32                        # window size (dst nodes per chunk)
NPAD = 6272                   # 128 * 49
NWIN = NPAD // W              # 196
HALF = 32768                  # int16 split of the gather table
NEG_SLOPE = 0.01
DUMP = NWIN                   # dump window id (pad chunks)
NBANK = 7                     # PSUM banks holding windows (196 = 7*28)


# ---------------------------------------------------------------- host prep
def _wrap16(flat):
    """dma/ap_gather idx layout: idx k at (partition k%16, col k//16),
    replicated across the 8 q7 cores (partition groups of 16)."""
    a = np.asarray(flat, np.int16).reshape(-1, 16).T
    return np.ascontiguousarray(np.tile(a, (8, 1)), dtype=np.int16)


def _prep(src, dst):
    """Split per core / window / stream; find shared per-window chunk counts."""
    cores = []
    for c in range(NCORES):
        n0 = c * NLOC
        e0, e1 = np.searchsorted(dst, [n0, n0 + NLOC])
        s_loc = src[e0:e1].astype(np.int64)
        d_loc = (dst[e0:e1] - n0).astype(np.int64)
        counts = np.bincount(d_loc // W, minlength=NWIN)
        ends = np.cumsum(counts)
        starts = ends - counts
        per_win = []
        for w in range(NWIN):
            sl = slice(starts[w], ends[w])
            s_w, d_w = s_loc[sl], d_loc[sl] - W * w
            m = s_w < HALF
            per_win.append(((s_w[m], d_w[m]), (s_w[~m] - HALF, d_w[~m])))
        cores.append(per_win)

    nch = np.zeros((NWIN, 2), np.int64)
    for per_win in cores:
        for w in range(NWIN):
            for st in (0, 1):
                nch[w, st] = max(nch[w, st],
                                 -(-len(per_win[w][st][0]) // 128))
    nch[nch.sum(1) == 0, 0] = 1      # >=1 chunk per window (PSUM init)
    return cores, nch


def _schedule(nch):
    """seq[pos] = (window, chunk_i, stream, start, stop) shared by all cores.

    PSUM start_tensor_calc zeroes the ENTIRE 2KB bank row (the "zero
    region") on the written partitions, so windows sharing a (bank,
    partition-group) row must form ONE accumulation group: start fires only
    on the row-group's very first chunk, stop on its last."""
    runs = []
    for st in (0, 1):
        lst = []
        for w in range(NWIN):
            for i in range(nch[w, st]):
                lst.append((w, i, st))
        n_sup = -(-len(lst) // K)
        lst += [(DUMP, 0, st)] * (n_sup * K - len(lst))
        runs.append(lst)
    s_lo, s_hi = len(runs[0]) // K, len(runs[1]) // K
    flat = runs[0] + runs[1]
    # row-group of window w: (bank w%7, partition-group (w//7)%4)
    first_pos, last_pos = {}, {}
    for pos, (w, i, st) in enumerate(flat):
        if w == DUMP:
            continue
        rg = (w % NBANK, (w // NBANK) % 4)
        if rg not in first_pos:
            first_pos[rg] = pos
        last_pos[rg] = pos
    seq = []
    for pos, (w, i, st) in enumerate(flat):
        if w == DUMP:
            seq.append((w, i, st, True, True))
        else:
            rg = (w % NBANK, (w // NBANK) % 4)
            seq.append((w, i, st, first_pos[rg] == pos, last_pos[rg] == pos))
    return seq, s_lo, s_hi


def _build_arrays(per_win, seq, s_lo, s_hi):
    """Per-core packed idx [S,128,128] f32, onehot mask [S,128,K*W] bf16."""
    S = s_lo + s_hi
    # pad slots must gather SOME valid row (mask=0 nullifies them); spread
    # them across the table — row-0 defaults serialize on one HBM bank
    idxg = np.empty((S, 4096), np.int64)
    for s in range(S):
        lim = HALF if s < s_lo else N - HALF
        idxg[s] = (np.arange(4096, dtype=np.int64) * 401 + s * 127) % lim
    mask = np.zeros((S, 128, K * W), BF16)
    aidx = np.full((S * K,), NWIN, np.int64)
    for pos, (w, i, st, _f, _l) in enumerate(seq):
        s, c = pos // K, pos % K
        if w == DUMP:
            continue
        ss, dd = per_win[w][st]
        ss, dd = ss[128 * i : 128 * i + 128], dd[128 * i : 128 * i + 128]
        ec = len(ss)
        idxg[s, c * 128 : c * 128 + ec] = ss
        mask[s, np.arange(ec), c * W + dd] = 1
        aidx[s * K + c] = w
    packed = np.empty((S, 128, 128), np.float32)
    for s in range(S):
        packed[s] = _wrap16(idxg[s]).view(np.float32)
    return packed, mask, _wrap16(aidx).view(np.float32)


# ------------------------------------------------------------- bass program
def _build_program(s_lo, s_hi, seq):
    import concourse.bacc as bacc
    import concourse.tile as tile
    import concourse.mybir as mybir
    from concourse import bass

    f32, i16, bf16 = mybir.dt.float32, mybir.dt.int16, mybir.dt.bfloat16
    AF = mybir.ActivationFunctionType
    OP = mybir.AluOpType
    S = s_lo + s_hi

    nc = bacc.Bacc("TRN2", target_bir_lowering=False, debug=False,
                   num_devices=NCORES, num_swdge_queues=4)
    hb_t = nc.dram_tensor("hb", [N, 128], bf16, kind="ExternalInput")
    hs_t = nc.dram_tensor("h_slice", [NPAD, F], f32, kind="ExternalInput")
    w_t = nc.dram_tensor("attn_w", [2 * F], f32, kind="ExternalInput")
    pk_t = nc.dram_tensor("packed", [S, 128, 128], f32, kind="ExternalInput")
    mk_t = nc.dram_tensor("mask", [S, 128, K * W], bf16, kind="ExternalInput")
    aw_t = nc.dram_tensor("aw", [128, S * K // 32], f32, kind="ExternalInput")
    out_t = nc.dram_tensor("out", [NPAD, F], f32, kind="ExternalOutput")
    dbg1_t = nc.dram_tensor("dbg1", [128, K], f32, kind="ExternalOutput")
    dbg2_t = nc.dram_tensor("dbg2", [128, K * W], f32, kind="ExternalOutput")
    dbg3_t = nc.dram_tensor("dbg3", [128, NBANK * 455], f32,
                            kind="ExternalOutput")
    adr_t = nc.dram_tensor("adr", [NPAD], f32, kind="Internal")
    ta_t = nc.dram_tensor("ta", [NWIN + 1, F], f32, kind="Internal")
    a2_t = nc.dram_tensor("a2", [S * K, W], bf16, kind="Internal")

    def bc_ap(tensor, offset, ap):
        return bass.AP(tensor=tensor, offset=offset, ap=ap)

    with tile.TileContext(nc) as tc:
        with tc.tile_pool(name="const", bufs=1) as const, \
             tc.tile_pool(name="pre", bufs=1) as pre, \
             tc.tile_pool(name="ps", bufs=1, space="PSUM") as ps:

            # ---------------- constants
            w1f = const.tile([128, F], f32)
            nc.gpsimd.dma_start(out=w1f[:], in_=bc_ap(w_t, 0, [[0, 128], [1, F]]))
            w2t = const.tile([128, F], f32)
            nc.gpsimd.dma_start(out=w2t[:], in_=bc_ap(w_t, F, [[0, 128], [1, F]]))
            w1b = const.tile([128, F], bf16)
            nc.vector.tensor_copy(w1b[:], w1f[:])

            # ---------------- preamble: a_dst table -> per-chunk A rows (bf16)
            with tc.tile_pool(name="pre2", bufs=1) as pre2:
                hs = pre2.tile([128, NPAD // 128, F], f32)
                nc.sync.dma_start(
                    out=hs[:], in_=hs_t[:].rearrange("(p t) f -> p t f", p=128))
                nc.vector.tensor_tensor(
                    out=hs[:], in0=hs[:],
                    in1=w2t[:, None, :].to_broadcast([128, NPAD // 128, F]),
                    op=OP.mult)
                a_sb = pre2.tile([128, NPAD // 128], f32)
                nc.vector.tensor_reduce(out=a_sb[:], in_=hs[:],
                                        axis=mybir.AxisListType.X, op=OP.add)
                nc.sync.dma_start(
                    out=adr_t[:].rearrange("(p t) -> p t", p=128), in_=a_sb[:])
                a_row = pre2.tile([1, NPAD], f32)
                nc.sync.dma_start(out=a_row[:],
                                  in_=bc_ap(adr_t, 0, [[0, 1], [1, NPAD]]))
                # ta_t row w = a_dst[32w .. 32w+32] (cols 32:64 unused);
                # row NWIN = zeros (dump chunks)
                nc.sync.dma_start(
                    out=ta_t[0:NWIN, 0:W],
                    in_=a_row[0:1, :].rearrange("p (w j) -> p w j", j=W))
                zrow = pre2.tile([1, NWIN + 1, W], f32)
                nc.vector.memset(zrow[:], 0.0)
                nc.sync.dma_start(out=ta_t[NWIN : NWIN + 1, 0:W],
                                  in_=zrow[0:1, 0, :])
                # cols W:2W are gathered (256B elems) but unused — keep them
                # initialized so CoreSim's finiteness checks pass
                nc.sync.dma_start(out=ta_t[:, W : 2 * W],
                                  in_=zrow[0:1, :, :])
                awi = pre2.tile([128, S * K // 32], f32)
                nc.sync.dma_start(out=awi[:], in_=aw_t[:])
                At = pre2.tile([128, S * K // 128, F], f32)
                nc.gpsimd.dma_gather(
                    out_ap=At[:], in_ap=ta_t[:],
                    idxs_ap=awi[:].bitcast(i16), num_idxs=S * K,
                    num_idxs_reg=S * K, elem_size=F, queue_num=0)
                A2 = pre2.tile([128, S * K // 128, W], bf16)
                nc.vector.tensor_copy(A2[:], At[:, :, 0:W])
                nc.sync.dma_start(
                    out=a2_t[:].rearrange("(c p) w -> p c w", p=128),
                    in_=A2[:])

            # ---------------- resident PSUM window accumulators
            # window w -> bank w%7, slot w//7: partition group (w//7)%4,
            # col group (w//7)//4. bank 7 = dump target for pad chunks.
            banks = [ps.tile([128, 512], f32, name=f"bank{b}", tag=f"bank{b}")
                     for b in range(8)]

            def bank_region(w):
                if w == DUMP:
                    return banks[7][0:32, 0:65], (0, 0)
                slot = w // NBANK
                p0, c0 = 32 * (slot % 4), 65 * (slot // 4)
                return banks[w % NBANK][p0 : p0 + 32, c0 : c0 + 65], (0, p0)

            # ---------------- super-steps (2-stage software pipeline)
            # stage A(s): gathers + DVE score prep + scalar Lrelu/Exp
            # stage B(s): Sp = P*mask, scatter matmuls — emitted one
            # iteration later so the scalar round trip never stalls DVE.
            from contextlib import ExitStack
            lctx = ExitStack()
            ldi = lctx.enter_context(tc.tile_pool(name="ldi", bufs=4))
            zp = lctx.enter_context(tc.tile_pool(name="zp", bufs=4))
            b3 = lctx.enter_context(tc.tile_pool(name="b3", bufs=3))
            med = lctx.enter_context(tc.tile_pool(name="med", bufs=3))
            mkp = lctx.enter_context(tc.tile_pool(name="mkp", bufs=4))

            stash = {}
            for it in range(S + 1):
                if it < S:
                    s = it
                    tab = hb_t[0:HALF, :] if s < s_lo else hb_t[HALF:N, :]
                    ld = ldi.tile([128, 128], f32, tag="ld")
                    nc.sync.dma_start(out=ld[:], in_=pk_t[s])
                    ig = ld[:].bitcast(i16)

                    Z = zp.tile([128, K, 128], bf16, tag="Z")
                    for q in range(4):
                        nc.gpsimd.dma_gather(
                            out_ap=Z[:, 8 * q : 8 * q + 8, :],
                            in_ap=tab,
                            idxs_ap=ig[:, 64 * q : 64 * q + 64],
                            num_idxs=1024, num_idxs_reg=1024, elem_size=128,
                            queue_num=q)

                    Mt = mkp.tile([128, K, W], bf16, tag="Mt")
                    nc.sync.dma_start(out=Mt[:],
                                      in_=mk_t[s].rearrange("p (c w) -> p c w", w=W))
                    # A[c, w] = a_dst[32*w_c + w], partition-replicated
                    A = med.tile([128, K, W], bf16, tag="A")
                    nc.sync.dma_start(
                        out=A[:],
                        in_=bc_ap(a2_t, s * K * W,
                                  [[0, 128], [W, K], [1, W]]))

                    # e = rowsum(Z * w1)
                    zw = med.tile([128, K, F], bf16, tag="zw")
                    nc.vector.tensor_tensor(
                        out=zw[:], in0=Z[:, :, 0:F],
                        in1=w1b[:, None, :].to_broadcast([128, K, F]),
                        op=OP.mult)
                    sC = med.tile([128, K], f32, tag="sC")
                    nc.vector.tensor_reduce(out=sC[:], in_=zw[:],
                                            axis=mybir.AxisListType.X,
                                            op=OP.add)
                    sCb = med.tile([128, K], bf16, tag="sCb")
                    nc.vector.tensor_copy(sCb[:], sC[:])
                    if s == 0:
                        stash_dbg = [sC]

                    # E = e + a_dst; P = exp(leaky_relu(E)) on Scalar engine
                    Emat = b3.tile([128, K, W], bf16, tag="Emat")
                    nc.vector.tensor_tensor(
                        out=Emat[:],
                        in0=sCb[:, :, None].to_broadcast([128, K, W]),
                        in1=A[:], op=OP.add)
                    El = b3.tile([128, K, W], bf16, tag="El")
                    nc.scalar.activation(out=El[:], in_=Emat[:], func=AF.Lrelu,
                                         alpha=NEG_SLOPE)
                    Pm = b3.tile([128, K, W], bf16, tag="Pm")
                    nc.scalar.activation(out=Pm[:], in_=El[:], func=AF.Exp)
                    stash[s] = (Z, Pm, Mt)

                if it >= 1:
                    s = it - 1
                    Z, Pm, Mt = stash.pop(s)
                    Sp = b3.tile([128, K, W], bf16, tag="Sp")
                    nc.vector.tensor_tensor(out=Sp[:], in0=Pm[:], in1=Mt[:],
                                            op=OP.mult)
                    if s == 0:
                        dsc = med.tile([128, K], f32, tag="dsc")
                        nc.vector.tensor_copy(dsc[:], stash_dbg[0])
                        nc.sync.dma_start(out=dbg1_t[:], in_=dsc[:])
                        dsp = med.tile([128, K * W], f32, tag="dsp")
                        nc.vector.tensor_copy(
                            dsp[:], Sp[:].rearrange("p c w -> p (c w)"))
                        nc.sync.dma_start(out=dbg2_t[:], in_=dsp[:])
                    for c in range(K):
                        w, _i, _st, first, last = seq[s * K + c]
                        reg, tpos = bank_region(w)
                        nc.tensor.matmul(out=reg, lhsT=Sp[:, c, :],
                                         rhs=Z[:, c, 0:F + 1],
                                         start=first, stop=last,
                                         tile_position=tpos)

            # ---------------- epilogue: drain banks, divide by denominator
            acc = pre.tile([128, NBANK, 28 // 4 * 65], f32)
            for b in range(NBANK):
                nc.scalar.copy(out=acc[:, b, :], in_=banks[b][:, 0 : 455])
            nc.sync.dma_start(out=dbg3_t[:],
                              in_=acc[:].rearrange("p b x -> p (b x)"))
            accv = acc[:].rearrange("p b (k x) -> p b k x", x=65)
            rmax = pre.tile([128, NBANK, 7], f32)
            nc.vector.tensor_scalar_max(rmax[:], accv[:, :, :, F], 1e-30)
            rcp = pre.tile([128, NBANK, 7], f32)
            nc.vector.reciprocal(rcp[:], rmax[:])
            nc.vector.tensor_tensor(
                out=accv[:, :, :, 0:F], in0=accv[:, :, :, 0:F],
                in1=rcp[:, :, :, None].to_broadcast([128, NBANK, 7, F]),
                op=OP.mult)
            # out rows in (b, k, g, r) device order; host inverse-permutes
            nc.sync.dma_start(
                out=out_t[:].rearrange("(b k g r) f -> (g r) b k f",
                                       b=NBANK, k=7, g=4),
                in_=accv[:, :, :, 0:F])
            lctx.close()
    nc.compile()
    return nc


_prog_cache = {}
_last_in_maps = None
_last_res = None


def kernel(h, attn_w, src, dst):
    from concourse.bass_utils import run_bass_kernel_spmd

    h = np.ascontiguousarray(h, dtype=np.float32)
    attn_w = np.ascontiguousarray(attn_w, dtype=np.float32)
    src = np.asarray(src, dtype=np.int32)
    dst = np.asarray(dst, dtype=np.int32)

    cores, nch = _prep(src, dst)
    seq, s_lo, s_hi = _schedule(nch)

    key = (s_lo, s_hi, tuple(seq))
    if key not in _prog_cache:
        _prog_cache[key] = _build_program(s_lo, s_hi, seq)
    nc = _prog_cache[key]

    # bf16 gather table: row n = [h[n] | 1.0 | zeros]; col 64 is the
    # softmax-denominator ones column
    hb = np.zeros((N, 128), BF16)
    hb[:, :F] = h
    hb[:, F] = 1.0

    in_maps = []
    for d in range(NCORES):
        n0 = d * NLOC
        packed, mask, aw = _build_arrays(cores[d], seq, s_lo, s_hi)
        h_slice = np.zeros((NPAD, F), np.float32)
        h_slice[:NLOC] = h[n0 : n0 + NLOC]
        in_maps.append({
            "hb": hb,
            "h_slice": h_slice,
            "attn_w": attn_w,
            "packed": packed,
            "mask": mask,
            "aw": aw,
        })

    global _last_in_maps, _last_res
    _last_in_maps = in_maps
    res = run_bass_kernel_spmd(nc, in_maps, list(range(NCORES)))
    _last_res = res
    # device rows are (bank, colgroup, partgroup, row): window w = 7*slot+b
    # with slot = 4*k+g lives at device row ((b*7+k)*4+g)*32+r
    b, k, g, r = np.meshgrid(np.arange(NBANK), np.arange(7), np.arange(4),
                             np.arange(32), indexing="ij")
    node = 32 * (NBANK * (4 * k + g) + b) + r
    inv = np.empty(NPAD, np.int64)
    inv[node.ravel()] = np.arange(NPAD)
    out = np.concatenate(
        [res.results[d]["out"][inv[:NLOC]] for d in range(NCORES)])
    return out.astype(np.float32)


if __name__ == "__main__":
    import reference

    inputs = reference.setup_inputs()
    inputs = {k: np.asarray(v) for k, v in inputs.items()}
    got = kernel(**inputs)
    exp = np.asarray(reference.reference(**inputs))
    denom = np.abs(exp).max()
    rel = np.abs(got - exp).max() / denom
    print("Relative error:", rel)


# revision 18
# speedup vs baseline: 1.2621x; 1.0911x over previous
"""GAT layer (edge softmax + weighted scatter) on 8 Trainium2 NeuronCores, v3.

Strategy (dst-range sharding, no collectives):
  - Nodes split into 8 contiguous dst ranges of 6250; dst is sorted, so each
    core owns a contiguous edge range and all of its destination segments.
  - Fixed 32-node window grid (196 windows/core). Edges of each window are
    split by src < 32768 (int16 gather limit) and chunked into <=128-edge
    chunks. Chunk counts per (window, stream) are maxed across cores so all
    8 cores share one compiled schedule; chunks run K=32 per super-step
    (lo-stream supersteps first, then hi).
  - Gather table is bf16 [N, 128]: row n = [h[n] (64) | 1.0 | zeros(63)].
    Col 64 provides the softmax-denominator ones column for free; bf16 makes
    the scatter matmuls single-pass (fp32 double-pumps the PE array).
  - Per super-step: 4x 1024-idx dma_gather pulls bf16 rows; scores
    e = rowsum(Z*w1) on DVE (bf16 mult + reduce); E = e + a_dst[window cols]
    (a_dst tile broadcast from a preamble-built table); leaky_relu and exp
    run on the Scalar engine (Lrelu + Exp); Sp = P * onehot-mask where the
    mask is HOST-built metadata DMA'd per super-step (no is_equal on DVE).
  - Scatter: matmul lhsT=Sp[:,c,:] [128,32] bf16, rhs=Z[:,c,0:65] bf16 into
    a PSUM-RESIDENT accumulator: 196 windows live across 7 PSUM banks
    (4 partition-groups x 7 col-groups of [32,65] each); start only on a
    window's first chunk, stop on its last. No per-run drains.
  - Epilogue: 7 whole-bank Scalar-engine drains to SBUF, divide features by
    the denominator column, one DMA writes the (window-permuted) output;
    the host inverse-permutes rows.
"""
import sys

sys.path.insert(0, "/opt/trn_rl_repo")

import numpy as np
import ml_dtypes

BF16 = ml_dtypes.bfloat16

N, F, E, NCORES = 50000, 64, 800000, 8
NLOC = N // NCORES            # 6250 nodes per core
K = 32                        # chunks per super-step
W = 32                        # window size (dst nodes per chunk)
NPAD = 6272                   # 128 * 49
NWIN = NPAD // W              # 196
HALF = 32768                  # int16 split of the gather table
NEG_SLOPE = 0.01
DUMP = NWIN                   # dump window id (pad chunks)
NBANK = 7                     # PSUM banks holding windows (196 = 7*28)


# ---------------------------------------------------------------- host prep
def _wrap16(flat):
    """dma/ap_gather idx layout: idx k at (partition k%16, col k//16),
    replicated across the 8 q7 cores (partition groups of 16)."""
    a = np.asarray(flat, np.int16).reshape(-1, 16).T
    return np.ascontiguousarray(np.tile(a, (8, 1)), dtype=np.int16)


def _prep(src, dst):
    """Split per core / window / stream; find shared per-window chunk counts."""
    cores = []
    for c in range(NCORES):
        n0 = c * NLOC
        e0, e1 = np.searchsorted(dst, [n0, n0 + NLOC])
        s_loc = src[e0:e1].astype(np.int64)
        d_loc = (dst[e0:e1] - n0).astype(np.int64)
        counts = np.bincount(d_loc // W, minlength=NWIN)
        ends = np.cumsum(counts)
        starts = ends - counts
        per_win = []
        for w in range(NWIN):
            sl = slice(starts[w], ends[w])
            s_w, d_w = s_loc[sl], d_loc[sl] - W * w
            m = s_w < HALF
            per_win.append(((s_w[m], d_w[m]), (s_w[~m] - HALF, d_w[~m])))
        cores.append(per_win)

    nch = np.zeros((NWIN, 2), np.int64)
    for per_win in cores:
        for w in range(NWIN):
            for st in (0, 1):
                nch[w, st] = max(nch[w, st],
                                 -(-len(per_win[w][st][0]) // 128))
    nch[nch.sum(1) == 0, 0] = 1      # >=1 chunk per window (PSUM init)
    return cores, nch


def _schedule(nch):
    """seq[pos] = (window, chunk_i, stream, start, stop) shared by all cores.

    PSUM start_tensor_calc zeroes the ENTIRE 2KB bank row (the "zero
    region") on the written partitions, so windows sharing a (bank,
    partition-group) row must form ONE accumulation group: start fires only
    on the row-group's very first chunk, stop on its last."""
    runs = []
    for st in (0, 1):
        lst = []
        for w in range(NWIN):
            for i in range(nch[w, st]):
                lst.append((w, i, st))
        n_sup = -(-len(lst) // K)
        lst += [(DUMP, 0, st)] * (n_sup * K - len(lst))
        runs.append(lst)
    s_lo, s_hi = len(runs[0]) // K, len(runs[1]) // K
    flat = runs[0] + runs[1]
    # row-group of window w: (bank w%7, partition-group (w//7)%4)
    first_pos, last_pos = {}, {}
    for pos, (w, i, st) in enumerate(flat):
        if w == DUMP:
            continue
        rg = (w % NBANK, (w // NBANK) % 4)
        if rg not in first_pos:
            first_pos[rg] = pos
        last_pos[rg] = pos
    seq = []
    for pos, (w, i, st) in enumerate(flat):
        if w == DUMP:
            seq.append((w, i, st, True, True))
        else:
            rg = (w % NBANK, (w // NBANK) % 4)
            seq.append((w, i, st, first_pos[rg] == pos, last_pos[rg] == pos))
    return seq, s_lo, s_hi


def _build_arrays(per_win, seq, s_lo, s_hi):
    """Per-core packed idx [S,128,128] f32, onehot mask [S,128,K*W] bf16."""
    S = s_lo + s_hi
    # pad slots must gather SOME valid row (mask=0 nullifies them); spread
    # them across the table — row-0 defaults serialize on one HBM bank
    idxg = np.empty((S, 4096), np.int64)
    for s in range(S):
        lim = HALF if s < s_lo else N - HALF
        idxg[s] = (np.arange(4096, dtype=np.int64) * 401 + s * 127) % lim
    mask = np.zeros((S, 128, K * W), BF16)
    aidx = np.full((S * K,), NWIN, np.int64)
    for pos, (w, i, st, _f, _l) in enumerate(seq):
        s, c = pos // K, pos % K
        if w == DUMP:
            continue
        ss, dd = per_win[w][st]
        ss, dd = ss[128 * i : 128 * i + 128], dd[128 * i : 128 * i + 128]
        ec = len(ss)
        idxg[s, c * 128 : c * 128 + ec] = ss
        mask[s, np.arange(ec), c * W + dd] = 1
        aidx[s * K + c] = w
    packed = np.empty((S, 128, 128), np.float32)
    for s in range(S):
        packed[s] = _wrap16(idxg[s]).view(np.float32)
    return packed, mask, _wrap16(aidx).view(np.float32)


# ------------------------------------------------------------- bass program
def _build_program(s_lo, s_hi, seq):
    import concourse.bacc as bacc
    import concourse.tile as tile
    import concourse.mybir as mybir
    from concourse import bass

    f32, i16, bf16 = mybir.dt.float32, mybir.dt.int16, mybir.dt.bfloat16
    AF = mybir.ActivationFunctionType
    OP = mybir.AluOpType
    S = s_lo + s_hi

    nc = bacc.Bacc("TRN2", target_bir_lowering=False, debug=False,
                   num_devices=NCORES, num_swdge_queues=4)
    hb_t = nc.dram_tensor("hb", [N, 128], bf16, kind="ExternalInput")
    hs_t = nc.dram_tensor("h_slice", [NPAD, F], f32, kind="ExternalInput")
    w_t = nc.dram_tensor("attn_w", [2 * F], f32, kind="ExternalInput")
    pk_t = nc.dram_tensor("packed", [S, 128, 128], f32, kind="ExternalInput")
    mk_t = nc.dram_tensor("mask", [S, 128, K * W], bf16, kind="ExternalInput")
    aw_t = nc.dram_tensor("aw", [128, S * K // 32], f32, kind="ExternalInput")
    out_t = nc.dram_tensor("out", [NPAD, F], f32, kind="ExternalOutput")
    adr_t = nc.dram_tensor("adr", [NPAD], f32, kind="Internal")
    ta_t = nc.dram_tensor("ta", [NWIN + 1, F], f32, kind="Internal")
    a2_t = nc.dram_tensor("a2", [S * K, W], bf16, kind="Internal")

    def bc_ap(tensor, offset, ap):
        return bass.AP(tensor=tensor, offset=offset, ap=ap)

    with tile.TileContext(nc) as tc:
        with tc.tile_pool(name="const", bufs=1) as const, \
             tc.tile_pool(name="pre", bufs=1) as pre, \
             tc.tile_pool(name="ps", bufs=1, space="PSUM") as ps:

            # ---------------- constants
            w1f = const.tile([128, F], f32)
            nc.gpsimd.dma_start(out=w1f[:], in_=bc_ap(w_t, 0, [[0, 128], [1, F]]))
            w2t = const.tile([128, F], f32)
            nc.gpsimd.dma_start(out=w2t[:], in_=bc_ap(w_t, F, [[0, 128], [1, F]]))
            w1b = const.tile([128, F], bf16)
            nc.vector.tensor_copy(w1b[:], w1f[:])

            # ---------------- preamble: a_dst table -> per-chunk A rows (bf16)
            with tc.tile_pool(name="pre2", bufs=1) as pre2:
                hs = pre2.tile([128, NPAD // 128, F], f32)
                nc.sync.dma_start(
                    out=hs[:], in_=hs_t[:].rearrange("(p t) f -> p t f", p=128))
                nc.vector.tensor_tensor(
                    out=hs[:], in0=hs[:],
                    in1=w2t[:, None, :].to_broadcast([128, NPAD // 128, F]),
                    op=OP.mult)
                a_sb = pre2.tile([128, NPAD // 128], f32)
                nc.vector.tensor_reduce(out=a_sb[:], in_=hs[:],
                                        axis=mybir.AxisListType.X, op=OP.add)
                nc.sync.dma_start(
                    out=adr_t[:].rearrange("(p t) -> p t", p=128), in_=a_sb[:])
                a_row = pre2.tile([1, NPAD], f32)
                nc.sync.dma_start(out=a_row[:],
                                  in_=bc_ap(adr_t, 0, [[0, 1], [1, NPAD]]))
                # ta_t row w = a_dst[32w .. 32w+32] (cols 32:64 unused);
                # row NWIN = zeros (dump chunks)
                nc.sync.dma_start(
                    out=ta_t[0:NWIN, 0:W],
                    in_=a_row[0:1, :].rearrange("p (w j) -> p w j", j=W))
                zrow = pre2.tile([128, W], f32)
                nc.vector.memset(zrow[:], 0.0)
                nc.sync.dma_start(out=ta_t[NWIN : NWIN + 1, 0:W],
                                  in_=zrow[0:1, :])
                # cols W:2W are gathered (256B elems) but unused — keep them
                # initialized so CoreSim's finiteness checks pass
                nc.sync.dma_start(
                    out=ta_t[:, W : 2 * W],
                    in_=zrow[0:1, None, :].to_broadcast([1, NWIN + 1, W]))
                awi = pre2.tile([128, S * K // 32], f32)
                nc.sync.dma_start(out=awi[:], in_=aw_t[:])
                At = pre2.tile([128, S * K // 128, F], f32)
                nc.gpsimd.dma_gather(
                    out_ap=At[:], in_ap=ta_t[:],
                    idxs_ap=awi[:].bitcast(i16), num_idxs=S * K,
                    num_idxs_reg=S * K, elem_size=F, queue_num=0)
                A2 = pre2.tile([128, S * K // 128, W], bf16)
                nc.vector.tensor_copy(A2[:], At[:, :, 0:W])
                nc.sync.dma_start(
                    out=a2_t[:].rearrange("(c p) w -> p c w", p=128),
                    in_=A2[:])

            # ---------------- resident PSUM window accumulators
            # window w -> bank w%7, slot w//7: partition group (w//7)%4,
            # col group (w//7)//4. bank 7 = dump target for pad chunks.
            banks = [ps.tile([128, 512], f32, name=f"bank{b}", tag=f"bank{b}")
                     for b in range(8)]

            def bank_region(w):
                if w == DUMP:
                    return banks[7][0:32, 0:65], (0, 0)
                slot = w // NBANK
                p0, c0 = 32 * (slot % 4), 65 * (slot // 4)
                return banks[w % NBANK][p0 : p0 + 32, c0 : c0 + 65], (0, p0)

            # ---------------- super-steps (2-stage software pipeline)
            # stage A(s): gathers + DVE score prep + scalar Lrelu/Exp
            # stage B(s): Sp = P*mask, scatter matmuls — emitted one
            # iteration later so the scalar round trip never stalls DVE.
            from contextlib import ExitStack
            lctx = ExitStack()
            ldi = lctx.enter_context(tc.tile_pool(name="ldi", bufs=4))
            zp = lctx.enter_context(tc.tile_pool(name="zp", bufs=4))
            b3 = lctx.enter_context(tc.tile_pool(name="b3", bufs=3))
            med = lctx.enter_context(tc.tile_pool(name="med", bufs=3))
            mkp = lctx.enter_context(tc.tile_pool(name="mkp", bufs=4))

            stash = {}
            for it in range(S + 1):
                if it < S:
                    s = it
                    tab = hb_t[0:HALF, :] if s < s_lo else hb_t[HALF:N, :]
                    ld = ldi.tile([128, 128], f32, tag="ld")
                    nc.sync.dma_start(out=ld[:], in_=pk_t[s])
                    ig = ld[:].bitcast(i16)

                    Z = zp.tile([128, K, 128], bf16, tag="Z")
                    for q in range(4):
                        nc.gpsimd.dma_gather(
                            out_ap=Z[:, 8 * q : 8 * q + 8, :],
                            in_ap=tab,
                            idxs_ap=ig[:, 64 * q : 64 * q + 64],
                            num_idxs=1024, num_idxs_reg=1024, elem_size=128,
                            queue_num=q)

                    Mt = mkp.tile([128, K, W], bf16, tag="Mt")
                    nc.sync.dma_start(out=Mt[:],
                                      in_=mk_t[s].rearrange("p (c w) -> p c w", w=W))
                    # A[c, w] = a_dst[32*w_c + w], partition-replicated
                    A = med.tile([128, K, W], bf16, tag="A")
                    nc.sync.dma_start(
                        out=A[:],
                        in_=bc_ap(a2_t, s * K * W,
                                  [[0, 128], [W, K], [1, W]]))

                    # e = rowsum(Z * w1) — DVE reduce accumulates fp32
                    # internally; bf16 output rounding is within tolerance
                    zw = med.tile([128, K, F], bf16, tag="zw")
                    nc.vector.tensor_tensor(
                        out=zw[:], in0=Z[:, :, 0:F],
                        in1=w1b[:, None, :].to_broadcast([128, K, F]),
                        op=OP.mult)
                    sCb = med.tile([128, K], bf16, tag="sCb")
                    with nc.allow_low_precision("bf16 scores within 2e-2 gate"):
                        nc.vector.tensor_reduce(out=sCb[:], in_=zw[:],
                                                axis=mybir.AxisListType.X,
                                                op=OP.add)

                    # E = e + a_dst; leaky = max(E, 0.01E) (ACT Copy shares
                    # Exp's table set — Lrelu does not and thrashes loads)
                    Emat = b3.tile([128, K, W], bf16, tag="Emat")
                    nc.vector.tensor_tensor(
                        out=Emat[:],
                        in0=sCb[:, :, None].to_broadcast([128, K, W]),
                        in1=A[:], op=OP.add)
                    El = b3.tile([128, K, W], bf16, tag="El")
                    nc.scalar.activation(out=El[:], in_=Emat[:], func=AF.Copy,
                                         scale=NEG_SLOPE)
                    nc.vector.tensor_tensor(out=El[:], in0=El[:], in1=Emat[:],
                                            op=OP.max)
                    Pm = b3.tile([128, K, W], bf16, tag="Pm")
                    nc.scalar.activation(out=Pm[:], in_=El[:], func=AF.Exp)
                    stash[s] = (Z, Pm, Mt)

                if it >= 1:
                    s = it - 1
                    Z, Pm, Mt = stash.pop(s)
                    Sp = b3.tile([128, K, W], bf16, tag="Sp")
                    nc.vector.tensor_tensor(out=Sp[:], in0=Pm[:], in1=Mt[:],
                                            op=OP.mult)
                    for c in range(K):
                        w, _i, _st, first, last = seq[s * K + c]
                        reg, tpos = bank_region(w)
                        nc.tensor.matmul(out=reg, lhsT=Sp[:, c, :],
                                         rhs=Z[:, c, 0:F + 1],
                                         start=first, stop=last,
                                         tile_position=tpos)

            # ---------------- epilogue: drain banks, divide by denominator
            acc = pre.tile([128, NBANK, 28 // 4 * 65], f32)
            for b in range(NBANK):
                nc.scalar.copy(out=acc[:, b, :], in_=banks[b][:, 0 : 455])
            accv = acc[:].rearrange("p b (k x) -> p b k x", x=65)
            rmax = pre.tile([128, NBANK, 7], f32)
            nc.vector.tensor_scalar_max(rmax[:], accv[:, :, :, F], 1e-30)
            rcp = pre.tile([128, NBANK, 7], f32)
            nc.vector.reciprocal(rcp[:], rmax[:])
            nc.vector.tensor_tensor(
                out=accv[:, :, :, 0:F], in0=accv[:, :, :, 0:F],
                in1=rcp[:, :, :, None].to_broadcast([128, NBANK, 7, F]),
                op=OP.mult)
            # out rows in (b, k, g, r) device order; host inverse-permutes
            nc.sync.dma_start(
                out=out_t[:].rearrange("(b k g r) f -> (g r) b k f",
                                       b=NBANK, k=7, g=4),
                in_=accv[:, :, :, 0:F])
            lctx.close()
    nc.compile()
    return nc


_prog_cache = {}
_last_in_maps = None
_last_res = None


def kernel(h, attn_w, src, dst):
    from concourse.bass_utils import run_bass_kernel_spmd

    h = np.ascontiguousarray(h, dtype=np.float32)
    attn_w = np.ascontiguousarray(attn_w, dtype=np.float32)
    src = np.asarray(src, dtype=np.int32)
    dst = np.asarray(dst, dtype=np.int32)

    cores, nch = _prep(src, dst)
    seq, s_lo, s_hi = _schedule(nch)

    key = (s_lo, s_hi, tuple(seq))
    if key not in _prog_cache:
        _prog_cache[key] = _build_program(s_lo, s_hi, seq)
    nc = _prog_cache[key]

    # bf16 gather table: row n = [h[n] | 1.0 | zeros]; col 64 is the
    # softmax-denominator ones column
    hb = np.zeros((N, 128), BF16)
    hb[:, :F] = h
    hb[:, F] = 1.0

    in_maps = []
    for d in range(NCORES):
        n0 = d * NLOC
        packed, mask, aw = _build_arrays(cores[d], seq, s_lo, s_hi)
        h_slice = np.zeros((NPAD, F), np.float32)
        h_slice[:NLOC] = h[n0 : n0 + NLOC]
        in_maps.append({
            "hb": hb,
            "h_slice": h_slice,
            "attn_w": attn_w,
            "packed": packed,
            "mask": mask,
            "aw": aw,
        })

    global _last_in_maps, _last_res
    _last_in_maps = in_maps
    res = run_bass_kernel_spmd(nc, in_maps, list(range(NCORES)))
    _last_res = res
    # device rows are (bank, colgroup, partgroup, row): window w = 7*slot+b
    # with slot = 4*k+g lives at device row ((b*7+k)*4+g)*32+r
    b, k, g, r = np.meshgrid(np.arange(NBANK), np.arange(7), np.arange(4),
                             np.arange(32), indexing="ij")
    node = 32 * (NBANK * (4 * k + g) + b) + r
    inv = np.empty(NPAD, np.int64)
    inv[node.ravel()] = np.arange(NPAD)
    out = np.concatenate(
        [res.results[d]["out"][inv[:NLOC]] for d in range(NCORES)])
    return out.astype(np.float32)


if __name__ == "__main__":
    import reference

    inputs = reference.setup_inputs()
    inputs = {k: np.asarray(v) for k, v in inputs.items()}
    got = kernel(**inputs)
    exp = np.asarray(reference.reference(**inputs))
    denom = np.abs(exp).max()
    rel = np.abs(got - exp).max() / denom
    print("Relative error:", rel)


# revision 21
# speedup vs baseline: 1.4317x; 1.1345x over previous
"""GAT layer (edge softmax + weighted scatter) on 8 Trainium2 NeuronCores, v3.

Strategy (dst-range sharding, no collectives):
  - Nodes split into 8 contiguous dst ranges of 6250; dst is sorted, so each
    core owns a contiguous edge range and all of its destination segments.
  - Fixed 32-node window grid (196 windows/core). Edges of each window are
    split by src < 32768 (int16 gather limit) and chunked into <=128-edge
    chunks. Chunk counts per (window, stream) are maxed across cores so all
    8 cores share one compiled schedule; chunks run K=32 per super-step
    (lo-stream supersteps first, then hi).
  - Gather table is bf16 [N, 128]: row n = [h[n] (64) | 1.0 | zeros(63)].
    Col 64 provides the softmax-denominator ones column for free; bf16 makes
    the scatter matmuls single-pass (fp32 double-pumps the PE array).
  - Per super-step: 4x 1024-idx dma_gather pulls bf16 rows; scores
    e = rowsum(Z*w1) on DVE (bf16 mult + reduce); E = e + a_dst[window cols]
    (a_dst tile broadcast from a preamble-built table); leaky_relu and exp
    run on the Scalar engine (Lrelu + Exp); Sp = P * onehot-mask where the
    mask is HOST-built metadata DMA'd per super-step (no is_equal on DVE).
  - Scatter: matmul lhsT=Sp[:,c,:] [128,32] bf16, rhs=Z[:,c,0:65] bf16 into
    a PSUM-RESIDENT accumulator: 196 windows live across 7 PSUM banks
    (4 partition-groups x 7 col-groups of [32,65] each); start only on a
    window's first chunk, stop on its last. No per-run drains.
  - Epilogue: 7 whole-bank Scalar-engine drains to SBUF, divide features by
    the denominator column, one DMA writes the (window-permuted) output;
    the host inverse-permutes rows.
"""
import sys

sys.path.insert(0, "/opt/trn_rl_repo")

import numpy as np
import ml_dtypes

BF16 = ml_dtypes.bfloat16

N, F, E, NCORES = 50000, 64, 800000, 8
NLOC = N // NCORES            # 6250 nodes per core
K = 32                        # chunks per super-step
W = 32                        # window size (dst nodes per chunk)
NPAD = 6272                   # 128 * 49
NWIN = NPAD // W              # 196
HALF = 32768                  # int16 split of the gather table
NEG_SLOPE = 0.01
DUMP = NWIN                   # dump window id (pad chunks)
NBANK = 7                     # PSUM banks holding windows (196 = 7*28)


# ---------------------------------------------------------------- host prep
def _wrap16(flat):
    """dma/ap_gather idx layout: idx k at (partition k%16, col k//16),
    replicated across the 8 q7 cores (partition groups of 16)."""
    a = np.asarray(flat, np.int16).reshape(-1, 16).T
    return np.ascontiguousarray(np.tile(a, (8, 1)), dtype=np.int16)


def _prep(src, dst):
    """Split per core / window / stream; find shared per-window chunk counts."""
    cores = []
    for c in range(NCORES):
        n0 = c * NLOC
        e0, e1 = np.searchsorted(dst, [n0, n0 + NLOC])
        s_loc = src[e0:e1].astype(np.int64)
        d_loc = (dst[e0:e1] - n0).astype(np.int64)
        counts = np.bincount(d_loc // W, minlength=NWIN)
        ends = np.cumsum(counts)
        starts = ends - counts
        per_win = []
        for w in range(NWIN):
            sl = slice(starts[w], ends[w])
            s_w, d_w = s_loc[sl], d_loc[sl] - W * w
            m = s_w < HALF
            per_win.append(((s_w[m], d_w[m]), (s_w[~m] - HALF, d_w[~m])))
        cores.append(per_win)

    nch = np.zeros((NWIN, 2), np.int64)
    for per_win in cores:
        for w in range(NWIN):
            for st in (0, 1):
                nch[w, st] = max(nch[w, st],
                                 -(-len(per_win[w][st][0]) // 128))
    nch[nch.sum(1) == 0, 0] = 1      # >=1 chunk per window (PSUM init)
    return cores, nch


def _schedule(nch):
    """seq[pos] = (window, chunk_i, stream, start, stop) shared by all cores.

    PSUM start_tensor_calc zeroes the ENTIRE 2KB bank row (the "zero
    region") on the written partitions, so windows sharing a (bank,
    partition-group) row must form ONE accumulation group: start fires only
    on the row-group's very first chunk, stop on its last."""
    runs = []
    for st in (0, 1):
        lst = []
        for w in range(NWIN):
            for i in range(nch[w, st]):
                lst.append((w, i, st))
        n_sup = -(-len(lst) // K)
        lst += [(DUMP, 0, st)] * (n_sup * K - len(lst))
        runs.append(lst)
    s_lo, s_hi = len(runs[0]) // K, len(runs[1]) // K
    flat = runs[0] + runs[1]
    # row-group of window w: (bank w%7, partition-group (w//7)%4)
    first_pos, last_pos = {}, {}
    for pos, (w, i, st) in enumerate(flat):
        if w == DUMP:
            continue
        rg = (w % NBANK, (w // NBANK) % 4)
        if rg not in first_pos:
            first_pos[rg] = pos
        last_pos[rg] = pos
    seq = []
    for pos, (w, i, st) in enumerate(flat):
        if w == DUMP:
            seq.append((w, i, st, True, True))
        else:
            rg = (w % NBANK, (w // NBANK) % 4)
            seq.append((w, i, st, first_pos[rg] == pos, last_pos[rg] == pos))
    return seq, s_lo, s_hi


def _build_arrays(per_win, seq, s_lo, s_hi):
    """Per-core packed idx [S,128,128] f32, onehot mask [S,128,K*W] bf16."""
    S = s_lo + s_hi
    # pad slots must gather SOME valid row (mask=0 nullifies them); spread
    # them across the table — row-0 defaults serialize on one HBM bank
    idxg = np.empty((S, 4096), np.int64)
    for s in range(S):
        lim = HALF if s < s_lo else N - HALF
        idxg[s] = (np.arange(4096, dtype=np.int64) * 401 + s * 127) % lim
    mask = np.zeros((S, 128, K * W), BF16)
    aidx = np.full((S * K,), NWIN, np.int64)
    for pos, (w, i, st, _f, _l) in enumerate(seq):
        s, c = pos // K, pos % K
        if w == DUMP:
            continue
        ss, dd = per_win[w][st]
        ss, dd = ss[128 * i : 128 * i + 128], dd[128 * i : 128 * i + 128]
        ec = len(ss)
        idxg[s, c * 128 : c * 128 + ec] = ss
        mask[s, np.arange(ec), c * W + dd] = 1
        aidx[s * K + c] = w
    packed = np.empty((S, 128, 128), np.float32)
    for s in range(S):
        packed[s] = _wrap16(idxg[s]).view(np.float32)
    return packed, mask, _wrap16(aidx).view(np.float32)


# ------------------------------------------------------------- bass program
def _build_program(s_lo, s_hi, seq):
    import concourse.bacc as bacc
    import concourse.tile as tile
    import concourse.mybir as mybir
    from concourse import bass

    f32, i16, bf16 = mybir.dt.float32, mybir.dt.int16, mybir.dt.bfloat16
    AF = mybir.ActivationFunctionType
    OP = mybir.AluOpType
    S = s_lo + s_hi

    nc = bacc.Bacc("TRN2", target_bir_lowering=False, debug=False,
                   num_devices=NCORES, num_swdge_queues=4)
    hb_t = nc.dram_tensor("hb", [N, 128], bf16, kind="ExternalInput")
    hs_t = nc.dram_tensor("h_slice", [NPAD, F], f32, kind="ExternalInput")
    w_t = nc.dram_tensor("attn_w", [2 * F], f32, kind="ExternalInput")
    pk_t = nc.dram_tensor("packed", [S, 128, 128], f32, kind="ExternalInput")
    mk_t = nc.dram_tensor("mask", [S, 128, K * W], bf16, kind="ExternalInput")
    aw_t = nc.dram_tensor("aw", [128, S * K // 32], f32, kind="ExternalInput")
    out_t = nc.dram_tensor("out", [NPAD, F], f32, kind="ExternalOutput")
    adr_t = nc.dram_tensor("adr", [NPAD], f32, kind="Internal")
    ta_t = nc.dram_tensor("ta", [NWIN + 1, F], f32, kind="Internal")
    a2_t = nc.dram_tensor("a2", [S * K, W], bf16, kind="Internal")

    def bc_ap(tensor, offset, ap):
        return bass.AP(tensor=tensor, offset=offset, ap=ap)

    with tile.TileContext(nc) as tc:
        with tc.tile_pool(name="const", bufs=1) as const, \
             tc.tile_pool(name="pre", bufs=1) as pre, \
             tc.tile_pool(name="ps", bufs=1, space="PSUM") as ps:

            # ---------------- constants
            w1f = const.tile([128, F], f32)
            nc.gpsimd.dma_start(out=w1f[:], in_=bc_ap(w_t, 0, [[0, 128], [1, F]]))
            w2t = const.tile([128, F], f32)
            nc.gpsimd.dma_start(out=w2t[:], in_=bc_ap(w_t, F, [[0, 128], [1, F]]))
            w1b = const.tile([128, F], bf16)
            nc.vector.tensor_copy(w1b[:], w1f[:])
            # physically replicated w1 over the chunk dim: keeps the zw
            # multiply's inputs step-1 contiguous so the DVE picks 2x mode
            # (a stride-0 broadcast AP drops it to 1x)
            w1r = const.tile([128, K, F], bf16)
            nc.vector.tensor_copy(
                w1r[:], w1b[:, None, :].to_broadcast([128, K, F]))

            # ---------------- preamble: a_dst table -> per-chunk A rows (bf16)
            with tc.tile_pool(name="pre2", bufs=1) as pre2:
                hs = pre2.tile([128, NPAD // 128, F], f32)
                nc.sync.dma_start(
                    out=hs[:], in_=hs_t[:].rearrange("(p t) f -> p t f", p=128))
                nc.vector.tensor_tensor(
                    out=hs[:], in0=hs[:],
                    in1=w2t[:, None, :].to_broadcast([128, NPAD // 128, F]),
                    op=OP.mult)
                a_sb = pre2.tile([128, NPAD // 128], f32)
                nc.vector.tensor_reduce(out=a_sb[:], in_=hs[:],
                                        axis=mybir.AxisListType.X, op=OP.add)
                nc.sync.dma_start(
                    out=adr_t[:].rearrange("(p t) -> p t", p=128), in_=a_sb[:])
                a_row = pre2.tile([1, NPAD], f32)
                nc.sync.dma_start(out=a_row[:],
                                  in_=bc_ap(adr_t, 0, [[0, 1], [1, NPAD]]))
                # ta_t row w = a_dst[32w .. 32w+32] (cols 32:64 unused);
                # row NWIN = zeros (dump chunks)
                nc.sync.dma_start(
                    out=ta_t[0:NWIN, 0:W],
                    in_=a_row[0:1, :].rearrange("p (w j) -> p w j", j=W))
                zrow = pre2.tile([128, W], f32)
                nc.vector.memset(zrow[:], 0.0)
                nc.sync.dma_start(out=ta_t[NWIN : NWIN + 1, 0:W],
                                  in_=zrow[0:1, :])
                # cols W:2W are gathered (256B elems) but unused — keep them
                # initialized so CoreSim's finiteness checks pass
                nc.sync.dma_start(
                    out=ta_t[:, W : 2 * W],
                    in_=zrow[0:1, None, :].to_broadcast([1, NWIN + 1, W]))
                awi = pre2.tile([128, S * K // 32], f32)
                nc.sync.dma_start(out=awi[:], in_=aw_t[:])
                At = pre2.tile([128, S * K // 128, F], f32)
                nc.gpsimd.dma_gather(
                    out_ap=At[:], in_ap=ta_t[:],
                    idxs_ap=awi[:].bitcast(i16), num_idxs=S * K,
                    num_idxs_reg=S * K, elem_size=F, queue_num=0)
                A2 = pre2.tile([128, S * K // 128, W], bf16)
                nc.vector.tensor_copy(A2[:], At[:, :, 0:W])
                nc.sync.dma_start(
                    out=a2_t[:].rearrange("(c p) w -> p c w", p=128),
                    in_=A2[:])

            # ---------------- resident PSUM window accumulators
            # window w -> bank w%7, slot w//7: partition group (w//7)%4,
            # col group (w//7)//4. bank 7 = dump target for pad chunks.
            banks = [ps.tile([128, 512], f32, name=f"bank{b}", tag=f"bank{b}")
                     for b in range(8)]

            def bank_region(w):
                if w == DUMP:
                    return banks[7][0:32, 0:65], (0, 0)
                slot = w // NBANK
                p0, c0 = 32 * (slot % 4), 65 * (slot // 4)
                return banks[w % NBANK][p0 : p0 + 32, c0 : c0 + 65], (0, p0)

            # ---------------- super-steps (2-stage software pipeline)
            # stage A(s): gathers + DVE score prep + scalar Lrelu/Exp
            # stage B(s): Sp = P*mask, scatter matmuls — emitted one
            # iteration later so the scalar round trip never stalls DVE.
            from contextlib import ExitStack
            lctx = ExitStack()
            ldi = lctx.enter_context(tc.tile_pool(name="ldi", bufs=6))
            zp = lctx.enter_context(tc.tile_pool(name="zp", bufs=5))
            b3 = lctx.enter_context(tc.tile_pool(name="b3", bufs=4))
            med = lctx.enter_context(tc.tile_pool(name="med", bufs=4))
            mkp = lctx.enter_context(tc.tile_pool(name="mkp", bufs=5))

            stash = {}
            for it in range(S + 1):
                if it < S:
                    s = it
                    tab = hb_t[0:HALF, :] if s < s_lo else hb_t[HALF:N, :]
                    ld = ldi.tile([128, 128], f32, tag="ld")
                    nc.sync.dma_start(out=ld[:], in_=pk_t[s])
                    ig = ld[:].bitcast(i16)

                    Z = zp.tile([128, K, 128], bf16, tag="Z")
                    for q in range(4):
                        nc.gpsimd.dma_gather(
                            out_ap=Z[:, 8 * q : 8 * q + 8, :],
                            in_ap=tab,
                            idxs_ap=ig[:, 64 * q : 64 * q + 64],
                            num_idxs=1024, num_idxs_reg=1024, elem_size=128,
                            queue_num=q)

                    Mt = mkp.tile([128, K, W], bf16, tag="Mt")
                    nc.sync.dma_start(out=Mt[:],
                                      in_=mk_t[s].rearrange("p (c w) -> p c w", w=W))
                    # A[c, w] = a_dst[32*w_c + w], partition-replicated
                    A = med.tile([128, K, W], bf16, tag="A")
                    nc.sync.dma_start(
                        out=A[:],
                        in_=bc_ap(a2_t, s * K * W,
                                  [[0, 128], [W, K], [1, W]]))

                    # e = rowsum(Z * w1) — DVE reduce accumulates fp32
                    # internally; bf16 output rounding is within tolerance
                    zw = med.tile([128, K, F], bf16, tag="zw")
                    nc.vector.tensor_tensor(
                        out=zw[:], in0=Z[:, :, 0:F], in1=w1r[:],
                        op=OP.mult)
                    sCb = med.tile([128, K], bf16, tag="sCb")
                    with nc.allow_low_precision("bf16 scores within 2e-2 gate"):
                        nc.vector.tensor_reduce(out=sCb[:], in_=zw[:],
                                                axis=mybir.AxisListType.X,
                                                op=OP.add)

                    # E = e + a_dst; leaky = max(E, 0.01E) (ACT Copy shares
                    # Exp's table set — Lrelu does not and thrashes loads)
                    Emat = b3.tile([128, K, W], bf16, tag="Emat")
                    nc.vector.tensor_tensor(
                        out=Emat[:],
                        in0=sCb[:, :, None].to_broadcast([128, K, W]),
                        in1=A[:], op=OP.add)
                    El = b3.tile([128, K, W], bf16, tag="El")
                    nc.scalar.activation(out=El[:], in_=Emat[:], func=AF.Copy,
                                         scale=NEG_SLOPE)
                    nc.vector.tensor_tensor(out=El[:], in0=El[:], in1=Emat[:],
                                            op=OP.max)
                    Pm = b3.tile([128, K, W], bf16, tag="Pm")
                    nc.scalar.activation(out=Pm[:], in_=El[:], func=AF.Exp)
                    stash[s] = (Z, Pm, Mt)

                if it >= 1:
                    s = it - 1
                    Z, Pm, Mt = stash.pop(s)
                    Sp = b3.tile([128, K, W], bf16, tag="Sp")
                    nc.vector.tensor_tensor(out=Sp[:], in0=Pm[:], in1=Mt[:],
                                            op=OP.mult)
                    for c in range(K):
                        w, _i, _st, first, last = seq[s * K + c]
                        reg, tpos = bank_region(w)
                        nc.tensor.matmul(out=reg, lhsT=Sp[:, c, :],
                                         rhs=Z[:, c, 0:F + 1],
                                         start=first, stop=last,
                                         tile_position=tpos)

            # ---------------- epilogue: drain banks, divide by denominator
            acc = pre.tile([128, NBANK, 28 // 4 * 65], f32)
            for b in range(NBANK):
                nc.scalar.copy(out=acc[:, b, :], in_=banks[b][:, 0 : 455])
            accv = acc[:].rearrange("p b (k x) -> p b k x", x=65)
            rmax = pre.tile([128, NBANK, 7], f32)
            nc.vector.tensor_scalar_max(rmax[:], accv[:, :, :, F], 1e-30)
            rcp = pre.tile([128, NBANK, 7], f32)
            nc.vector.reciprocal(rcp[:], rmax[:])
            nc.vector.tensor_tensor(
                out=accv[:, :, :, 0:F], in0=accv[:, :, :, 0:F],
                in1=rcp[:, :, :, None].to_broadcast([128, NBANK, 7, F]),
                op=OP.mult)
            # out rows in (b, k, g, r) device order; host inverse-permutes
            nc.sync.dma_start(
                out=out_t[:].rearrange("(b k g r) f -> (g r) b k f",
                                       b=NBANK, k=7, g=4),
                in_=accv[:, :, :, 0:F])
            lctx.close()
    nc.compile()
    return nc


_prog_cache = {}
_last_in_maps = None
_last_res = None


def kernel(h, attn_w, src, dst):
    from concourse.bass_utils import run_bass_kernel_spmd

    h = np.ascontiguousarray(h, dtype=np.float32)
    attn_w = np.ascontiguousarray(attn_w, dtype=np.float32)
    src = np.asarray(src, dtype=np.int32)
    dst = np.asarray(dst, dtype=np.int32)

    cores, nch = _prep(src, dst)
    seq, s_lo, s_hi = _schedule(nch)

    key = (s_lo, s_hi, tuple(seq))
    if key not in _prog_cache:
        _prog_cache[key] = _build_program(s_lo, s_hi, seq)
    nc = _prog_cache[key]

    # bf16 gather table: row n = [h[n] | 1.0 | zeros]; col 64 is the
    # softmax-denominator ones column
    hb = np.zeros((N, 128), BF16)
    hb[:, :F] = h
    hb[:, F] = 1.0

    in_maps = []
    for d in range(NCORES):
        n0 = d * NLOC
        packed, mask, aw = _build_arrays(cores[d], seq, s_lo, s_hi)
        h_slice = np.zeros((NPAD, F), np.float32)
        h_slice[:NLOC] = h[n0 : n0 + NLOC]
        in_maps.append({
            "hb": hb,
            "h_slice": h_slice,
            "attn_w": attn_w,
            "packed": packed,
            "mask": mask,
            "aw": aw,
        })

    global _last_in_maps, _last_res
    _last_in_maps = in_maps
    res = run_bass_kernel_spmd(nc, in_maps, list(range(NCORES)))
    _last_res = res
    # device rows are (bank, colgroup, partgroup, row): window w = 7*slot+b
    # with slot = 4*k+g lives at device row ((b*7+k)*4+g)*32+r
    b, k, g, r = np.meshgrid(np.arange(NBANK), np.arange(7), np.arange(4),
                             np.arange(32), indexing="ij")
    node = 32 * (NBANK * (4 * k + g) + b) + r
    inv = np.empty(NPAD, np.int64)
    inv[node.ravel()] = np.arange(NPAD)
    out = np.concatenate(
        [res.results[d]["out"][inv[:NLOC]] for d in range(NCORES)])
    return out.astype(np.float32)


if __name__ == "__main__":
    import reference

    inputs = reference.setup_inputs()
    inputs = {k: np.asarray(v) for k, v in inputs.items()}
    got = kernel(**inputs)
    exp = np.asarray(reference.reference(**inputs))
    denom = np.abs(exp).max()
    rel = np.abs(got - exp).max() / denom
    print("Relative error:", rel)
